# revision 3
# baseline (speedup 1.0000x reference)
"""DeepSeekV3 MLA prefill kernel for 8 TRN2 NeuronCores.

Sharding: batch x query-strips. Cores 0-3 handle batch 0, cores 4-7
batch 1; core (b, c) owns 4 query strips of 128 tokens at positions
(4j+c)*128, j=0..3. Every core runs the SAME program (SPMD); the
per-core causal structure lives entirely in input data (gathered
hidden columns, cos/sin tables, one multiplicative 0/1 mask set).

Pipeline per core (feature-major, fp16 matmuls, fp32 PSUM):
  S1 q_latentT = Wq_down.T @ hiddenT[:, qpos]          [1536, 512]
  S2 kvdrT     = Wkv_down.T @ hiddenT (k-streamed)     [576, 2048]
  S3 RMSNorm both latents (ones-matmul partition reduction,
     ones-row matmul broadcast), k-rope
  S4 qT = Wq_up.T @ q_latn (rope cols host-permuted), q-rope
  S5 per head-group: V (token-major); per head: k_nopeT
  S6 per head: TRANSPOSED-score attention: scores[k,q] tiles from
     K-stationary matmuls, exp straight from PSUM (no max
     subtraction; scores bounded ~5.5), 0/1 mask multiply on the
     diagonal block, DVE-accumulated denominators, PV accumulates
     OT[dv,q] without any transposes.
  S7 outT = Wo.T @ OT (accumulate over heads) -> fp32 output
"""

import numpy as np

B, S, D = 2, 2048, 2048
H = 16
NOPE, ROPE, DV = 128, 64, 128
DQK = NOPE + ROPE
QLR, KVLR = 1536, 512
SCALE = 1.0 / float(np.sqrt(DQK))
EPS = 1e-5

NSTRIP = 4          # query strips per core
QB = 128            # strip width (= partition tile)
NQ = NSTRIP * QB    # 512 query tokens per core
KC = 512            # k chunk width in S2/S5
NKT = S // 128      # 16 k token-tiles
F16 = np.float16

_COMPILED = None


# ---------------------------------------------------------------------------
# walrus workaround: this build accepts only ONE sync-wait per instruction
# ---------------------------------------------------------------------------

def _apply_tile_patch():
    import concourse.mybir as mybir
    import concourse.tile as tile
    from concourse.vector_clock import ScopedClock

    if getattr(tile.TileContext, '_mla_patched', False):
        return

    def _patched_drain_and_barrier(self, tick_clock, wait_clock):
        nc = self.nc
        probe = nc.sync.nop(nofuse=True, hint="tile_drain_waits")
        wait_clock.add_sem_waits(
            probe.ins, ScopedClock({None: tick_clock.global_clock}))
        waits = list(probe.ins.sync_info.on_wait) if probe.ins.sync_info else []
        if probe.ins.sync_info:
            probe.ins.sync_info.on_wait[:] = waits[:1]
        rest = waits[1:]
        while rest:
            chunk, rest = rest[:1], rest[1:]
            inst = nc.sync.nop(nofuse=True, hint="tile_drain_waits")
            inst.ins.sync_info = mybir.SyncInfo(on_wait=list(chunk), on_update=[])
        nc.sync.drain()
        nc.all_engine_barrier()
        assert self.sems is not None
        popped = nc._tile_sem_poison_stack.pop()
        assert popped is self._sem_poison
        nc.clear_and_free_semaphores(list(self.sems.allocated().values()))
        nc.all_engine_barrier()

    tile.TileContext._drain_and_barrier = _patched_drain_and_barrier
    tile.TileContext._mla_patched = True


def _split_multiwait_instructions(nc):
    import concourse.mybir as mybir
    n_split = 0
    for bb in nc.m.functions[0].blocks:
        insert_at = []
        for idx, inst in enumerate(bb.instructions):
            si = inst.sync_info
            waits = list(si.on_wait) if si is not None else []
            if len(waits) > 1:
                nops = []
                for w in waits[:-1]:
                    nop = mybir.InstNoOp(
                        name=nc.get_next_instruction_name(),
                        engine=inst.engine, ins=[], outs=[], hint="split_wait")
                    nop.sync_info = mybir.SyncInfo(on_wait=[w], on_update=[])
                    nc.register_instruction(nop)
                    nops.append(nop)
                si.on_wait[:] = waits[-1:]
                insert_at.append((idx, nops))
                n_split += 1
        if insert_at:
            old = list(bb.instructions)
            ins_map = dict(insert_at)
            new_insts = []
            for idx, inst in enumerate(old):
                if idx in ins_map:
                    new_insts.extend(ins_map[idx])
                new_insts.append(inst)
            bb.instructions[:] = new_insts
    return n_split


# ---------------------------------------------------------------------------
# device program
# ---------------------------------------------------------------------------

def _build_nc():
    import concourse.bass as bass
    import concourse.mybir as mybir
    import concourse.tile as tile

    DT = mybir.dt
    AF = mybir.ActivationFunctionType
    OP = mybir.AluOpType

    nc = bass.Bass()

    def param(name, shape, dt=DT.float16):
        return nc.declare_dram_parameter(name, list(shape), dt, isOutput=False)

    NKD = D // 128      # 16
    NKQ = QLR // 128    # 12
    NKV = KVLR // 128   # 4
    NMQ = H * 192 // 128  # 24

    hTq_d = param('hTq', [NKD, 128, NQ])
    hT_d = param('hT', [NKD, 128, S])
    wqd_d = param('wqd', [NKQ, 128, NKD * 128])      # [m][p][k*128+c]
    wkvd_d = param('wkvd', [NKD, 128, KVLR + ROPE])
    wqu_d = param('wqu', [NMQ, 128, NKQ * 128])      # [m][p][k*128+c]
    wkvuk_d = param('wkvuk', [NKV, 128, H * NOPE])
    wkvuv_d = param('wkvuv', [4, 128, NKV * 512])    # [g][p][k*512+c]
    wo_d = param('wo', [NKD, 128, H * 128])          # [m][p][h*128+c]
    cosq_d = param('cosq', [128, NQ])
    ssinq_d = param('ssinq', [128, NQ])
    cosk_d = param('cosk', [64, S])
    ssink_d = param('ssink', [64, S])
    binm_d = param('binm', [128, 4 * 128])           # [kp][m*128+qf] 0/1
    out_d = nc.declare_dram_parameter(
        'outT', [NKD, 128, NQ], DT.float32, isOutput=True)

    with tile.TileContext(nc) as tc:
        with (
            tc.tile_pool(name='const', bufs=1) as const,
            tc.tile_pool(name='persist', bufs=1) as persist,
        ):
            ones = const.tile([128, 1], DT.float16)
            nc.vector.memset(ones[:], 1.0)
            onesr = const.tile([1, 128], DT.float16)
            nc.vector.memset(onesr[:], 1.0)
            epsb1 = const.tile([1, 1], DT.float32)
            nc.vector.memset(epsb1[:], EPS)
            binm = const.tile([128, 4, 128], DT.float16)
            nc.sync.dma_start(out=binm[:], in_=binm_d[:].rearrange(
                "p (m c) -> p m c", m=4))
            cosq = const.tile([128, NQ], DT.float16)
            nc.sync.dma_start(out=cosq[:], in_=cosq_d[:])
            ssinq = const.tile([128, NQ], DT.float16)
            nc.sync.dma_start(out=ssinq[:], in_=ssinq_d[:])

            latqn = [persist.tile([128, NQ], DT.float16, tag=f'latqn{t}', name=f'latqn{t}')
                     for t in range(NKQ)]
            latkvn = [persist.tile([128, S], DT.float16, tag=f'latkvn{t}', name=f'latkvn{t}')
                      for t in range(NKV)]
            krope = persist.tile([128, S], DT.float16, tag='krope')

            # ======== phase A: S1/S2/S3 (raw latents live only here) ========
            with (
                tc.tile_pool(name='hxp', bufs=1) as hxp,
                tc.tile_pool(name='aw', bufs=2) as aw,
                tc.tile_pool(name='hxs', bufs=3) as hxs,
                tc.tile_pool(name='a2', bufs=2) as a2,
                tc.tile_pool(name='pa', bufs=2, space='PSUM') as pa,
                tc.tile_pool(name='ps2', bufs=1, space='PSUM') as ps2,
                tc.tile_pool(name='pn', bufs=1, space='PSUM') as pn,
            ):
                # S1: hq resident, wqd streamed per m (contiguous tiles)
                hq = [hxp.tile([128, NQ], DT.float16, tag=f'hq{k}', name=f'hq{k}')
                      for k in range(NKD)]
                for k in range(NKD):
                    nc.sync.dma_start(out=hq[k][:], in_=hTq_d[k, :, :])
                # wkvd resident (needed for S2) - start DMAs early
                wkvd = [hxp.tile([128, KVLR + ROPE], DT.float16, tag=f'wkvd{k}', name=f'wkvd{k}')
                        for k in range(NKD)]
                for k in range(NKD):
                    nc.sync.dma_start(out=wkvd[k][:], in_=wkvd_d[k, :, :])

                for m in range(NKQ):
                    wq = aw.tile([128, NKD, 128], DT.float16, tag='wqdm')
                    nc.sync.dma_start(
                        out=wq[:],
                        in_=wqd_d[m].rearrange("p (k c) -> p k c", k=NKD))
                    ps = pa.tile([128, KC], DT.float32, tag='mm')
                    for k in range(NKD):
                        nc.tensor.matmul(ps[:], wq[:, k, :], hq[k][:],
                                         start=(k == 0), stop=(k == NKD - 1))
                    if m % 2 == 0:
                        nc.scalar.copy(latqn[m][:], ps[:])
                    else:
                        nc.vector.tensor_copy(latqn[m][:], ps[:])

                # q norm: sum of squares via ones-matmul
                psq = pn.tile([128, NQ], DT.float32, tag='nrm')
                for t in range(NKQ):
                    sq = a2.tile([128, NQ], DT.float16, tag='sq')
                    nc.scalar.activation(sq[:], latqn[t][:], AF.Square)
                    nc.tensor.matmul(psq[0:1, :], ones[:], sq[:],
                                     start=(t == 0), stop=(t == NKQ - 1))
                rowq = a2.tile([1, NQ], DT.float32, tag='rowq')
                nc.vector.tensor_copy(rowq[:], psq[0:1, :])
                sqq = a2.tile([1, NQ], DT.float32, tag='sqq')
                nc.scalar.activation(sqq[:], rowq[:], AF.Sqrt,
                                     bias=epsb1[:], scale=1.0 / QLR)
                rq = a2.tile([1, NQ], DT.float32, tag='rq')
                nc.vector.reciprocal(rq[:], sqq[:])
                rqh = a2.tile([1, NQ], DT.float16, tag='rqh')
                nc.vector.tensor_copy(rqh[:], rq[:])
                # broadcast via ones-row matmul, then scale latqn
                rqp = pa.tile([128, NQ], DT.float32, tag='mm')
                nc.tensor.matmul(rqp[:], onesr[:], rqh[:], start=True, stop=True)
                rqb = a2.tile([128, NQ], DT.float16, tag='rqb')
                nc.scalar.copy(rqb[:], rqp[:])
                for t in range(NKQ):
                    nc.vector.tensor_tensor(out=latqn[t][:], in0=latqn[t][:],
                                            in1=rqb[:], op=OP.mult)

                # S2: k-streamed, 5 concurrent PSUM chains per 512-chunk
                for n in range(S // KC):
                    cps = [ps2.tile([128, KC], DT.float32, tag=f'c{m}', name=f'c{m}')
                           for m in range(NKV)]
                    rps = ps2.tile([64, KC], DT.float32, tag='c4', name='c4')
                    for k in range(NKD):
                        hxn = hxs.tile([128, KC], DT.float16, tag='hxn')
                        nc.sync.dma_start(out=hxn[:],
                                          in_=hT_d[k, :, n * KC:(n + 1) * KC])
                        for m in range(NKV):
                            nc.tensor.matmul(cps[m][:],
                                             wkvd[k][:, m * 128:(m + 1) * 128],
                                             hxn[:],
                                             start=(k == 0), stop=(k == NKD - 1))
                        nc.tensor.matmul(rps[:], wkvd[k][:, KVLR:KVLR + ROPE],
                                         hxn[:],
                                         start=(k == 0), stop=(k == NKD - 1))
                    for m in range(NKV):
                        if m % 2 == 0:
                            nc.scalar.copy(latkvn[m][:, n * KC:(n + 1) * KC],
                                           cps[m][:])
                        else:
                            nc.vector.tensor_copy(
                                latkvn[m][:, n * KC:(n + 1) * KC], cps[m][:])
                    nc.scalar.copy(krope[0:64, n * KC:(n + 1) * KC], rps[:])

                    # kv norm for this chunk
                    psk = pn.tile([128, KC], DT.float32, tag='nrm')
                    for t in range(NKV):
                        sk = a2.tile([128, KC], DT.float16, tag='sk')
                        nc.scalar.activation(
                            sk[:], latkvn[t][:, n * KC:(n + 1) * KC], AF.Square)
                        nc.tensor.matmul(psk[0:1, :], ones[:], sk[:],
                                         start=(t == 0), stop=(t == NKV - 1))
                    rowk = a2.tile([1, KC], DT.float32, tag='rowk')
                    nc.vector.tensor_copy(rowk[:], psk[0:1, :])
                    sqk = a2.tile([1, KC], DT.float32, tag='sqk')
                    nc.scalar.activation(sqk[:], rowk[:], AF.Sqrt,
                                         bias=epsb1[:], scale=1.0 / KVLR)
                    rk = a2.tile([1, KC], DT.float32, tag='rk')
                    nc.vector.reciprocal(rk[:], sqk[:])
                    rkh = a2.tile([1, KC], DT.float16, tag='rkh')
                    nc.vector.tensor_copy(rkh[:], rk[:])
                    rkp = pa.tile([128, KC], DT.float32, tag='mm')
                    nc.tensor.matmul(rkp[:], onesr[:], rkh[:],
                                     start=True, stop=True)
                    rkb = a2.tile([128, KC], DT.float16, tag='rkb')
                    nc.scalar.copy(rkb[:], rkp[:])
                    for t in range(NKV):
                        nc.vector.tensor_tensor(
                            out=latkvn[t][:, n * KC:(n + 1) * KC],
                            in0=latkvn[t][:, n * KC:(n + 1) * KC],
                            in1=rkb[:], op=OP.mult)

                # k rope (krope rows 0:64, then duplicate into 64:128)
                cosk = a2.tile([64, S], DT.float16, tag='cosk')
                nc.sync.dma_start(out=cosk[:], in_=cosk_d[:])
                ssink = a2.tile([64, S], DT.float16, tag='ssink')
                nc.sync.dma_start(out=ssink[:], in_=ssink_d[:])
                xsk = a2.tile([64, S], DT.float16, tag='xsk')
                nc.vector.tensor_copy(xsk[0:32, :], krope[32:64, :])
                nc.vector.tensor_copy(xsk[32:64, :], krope[0:32, :])
                nc.vector.tensor_tensor(out=xsk[:], in0=xsk[:], in1=ssink[:],
                                        op=OP.mult)
                nc.vector.tensor_tensor(out=krope[0:64, :], in0=krope[0:64, :],
                                        in1=cosk[:], op=OP.mult)
                nc.vector.tensor_tensor(out=krope[0:64, :], in0=krope[0:64, :],
                                        in1=xsk[:], op=OP.add)
                nc.sync.dma_start(out=krope[64:128, :], in_=krope[0:64, :])

            # ======== phases B/C/D in one scope ============================
            with (
                tc.tile_pool(name='qtot', bufs=1) as qtot,
                tc.tile_pool(name='s4w', bufs=3) as s4w,
                tc.tile_pool(name='kvw', bufs=1) as kvw,
                tc.tile_pool(name='wvp', bufs=2) as wvp,
                tc.tile_pool(name='vng', bufs=1) as vng,
                tc.tile_pool(name='att', bufs=2) as att,
                tc.tile_pool(name='ett', bufs=3) as ett,
                tc.tile_pool(name='sml', bufs=2) as sml,
                tc.tile_pool(name='s7w', bufs=3) as s7w,
                tc.tile_pool(name='s7o', bufs=3) as s7o,
                tc.tile_pool(name='pmm', bufs=2, space='PSUM') as pmm,
                tc.tile_pool(name='pqk', bufs=2, space='PSUM') as pqk,
                tc.tile_pool(name='pot', bufs=2, space='PSUM') as pot,
                tc.tile_pool(name='pds', bufs=2, space='PSUM') as pds,
            ):
                # early weight DMAs for phase C (land during S4)
                wkvuk = [kvw.tile([128, H * NOPE], DT.float16, tag=f'wkvuk{k}', name=f'wkvuk{k}')
                         for k in range(NKV)]
                for k in range(NKV):
                    nc.sync.dma_start(out=wkvuk[k][:], in_=wkvuk_d[k, :, :])

                qT = [qtot.tile([128, NQ], DT.float16, tag=f'qT{t}', name=f'qT{t}')
                      for t in range(NMQ)]
                OT = [qtot.tile([128, NQ], DT.float16, tag=f'OT{h}', name=f'OT{h}')
                      for h in range(H)]
                v4 = [vng.tile([128, 512], DT.float16, tag=f'v4_{tb}', name=f'v4_{tb}')
                      for tb in range(NKT)]

                # ---- S4: q up-projection + q-rope -------------------------
                for m in range(NMQ):
                    wq = s4w.tile([128, NKQ, 128], DT.float16, tag='wqu')
                    nc.sync.dma_start(
                        out=wq[:],
                        in_=wqu_d[m].rearrange("p (k c) -> p k c", k=NKQ))
                    ps = pmm.tile([128, NQ], DT.float32, tag='mm')
                    for k in range(NKQ):
                        nc.tensor.matmul(ps[:], wq[:, k, :], latqn[k][:],
                                         start=(k == 0), stop=(k == NKQ - 1))
                    if m % 2 == 0:
                        nc.scalar.copy(qT[m][:], ps[:])
                    else:
                        nc.vector.tensor_copy(qT[m][:], ps[:])
                    if m >= 16:
                        xs = att.tile([128, NQ], DT.float16, tag='xs')
                        for half in range(4):
                            src = [32, 0, 96, 64][half]
                            nc.vector.tensor_copy(xs[half * 32:(half + 1) * 32, :],
                                                  qT[m][src:src + 32, :])
                        nc.vector.tensor_tensor(out=xs[:], in0=xs[:], in1=ssinq[:],
                                                op=OP.mult)
                        nc.vector.tensor_tensor(out=qT[m][:], in0=qT[m][:],
                                                in1=cosq[:], op=OP.mult)
                        nc.vector.tensor_tensor(out=qT[m][:], in0=qT[m][:],
                                                in1=xs[:], op=OP.add)

                # ---- S5+S6 per head, software-pipelined -------------------
                def emit_v4_group(g):
                    wv = wvp.tile([128, NKV, 512], DT.float16, tag='wv')
                    nc.sync.dma_start(
                        out=wv[:],
                        in_=wkvuv_d[g].rearrange("p (k c) -> p k c", k=NKV))
                    for tb in range(NKT):
                        ps = pmm.tile([128, 512], DT.float32, tag='mm')
                        for k in range(NKV):
                            nc.tensor.matmul(
                                ps[:], latkvn[k][:, tb * 128:(tb + 1) * 128],
                                wv[:, k, :], start=(k == 0),
                                stop=(k == NKV - 1))
                        if tb % 2 == 0:
                            nc.scalar.copy(v4[tb][:], ps[:])
                        else:
                            nc.vector.tensor_copy(v4[tb][:], ps[:])

                def emit_knope(h):
                    kn = att.tile([128, S], DT.float16, tag='kn')
                    for n_ in range(S // KC):
                        ps = pmm.tile([128, KC], DT.float32, tag='mm')
                        for k in range(NKV):
                            nc.tensor.matmul(
                                ps[:], wkvuk[k][:, h * 128:(h + 1) * 128],
                                latkvn[k][:, n_ * KC:(n_ + 1) * KC],
                                start=(k == 0), stop=(k == NKV - 1))
                        if n_ % 2 == 0:
                            nc.scalar.copy(kn[:, n_ * KC:(n_ + 1) * KC], ps[:])
                        else:
                            nc.vector.tensor_copy(kn[:, n_ * KC:(n_ + 1) * KC],
                                                  ps[:])
                    return kn

                # per-head state kept across the pipelined loop
                pend = {}   # h -> (ot_ps, ds_ps, dacc)

                def emit_qk(h, kn, s, ET):
                    qn = qT[h]
                    qr = qT[16 + h // 2]
                    qro = 64 * (h % 2)
                    jmin = s // 4
                    q0 = jmin * QB
                    N = NQ - q0
                    ps = pqk.tile([128, 512], DT.float32, tag='qk')
                    nc.tensor.matmul(ps[:, 0:N], kn[:, s * 128:(s + 1) * 128],
                                     qn[:, q0:NQ], start=True, stop=False)
                    nc.tensor.matmul(ps[:, 0:N],
                                     krope[qro:qro + 64, s * 128:(s + 1) * 128],
                                     qr[qro:qro + 64, q0:NQ],
                                     start=False, stop=True)
                    # exp from PSUM (scores bounded; no max subtraction)
                    nc.scalar.activation(ET[:, 0:N], ps[:, 0:N], AF.Exp,
                                         scale=SCALE)
                    # zero the non-causal part of the diagonal strip
                    nc.vector.tensor_tensor(out=ET[:, 0:QB], in0=ET[:, 0:QB],
                                            in1=binm[:, s % 4, :], op=OP.mult)
                    return ps

                def emit_pv(h, s, ET, ot_ps, dacc):
                    jmin = s // 4
                    q0 = jmin * QB
                    N = NQ - q0
                    nc.tensor.matmul(ot_ps[:, q0:NQ],
                                     v4[s][:, (h % 4) * 128:(h % 4 + 1) * 128],
                                     ET[:, 0:N],
                                     start=(s == 0), stop=(s == NKT - 1))
                    if s == 0:
                        nc.vector.tensor_copy(dacc[:], ET[:, 0:NQ])
                    else:
                        nc.vector.tensor_tensor(out=dacc[:, q0:NQ],
                                                in0=dacc[:, q0:NQ],
                                                in1=ET[:, 0:N], op=OP.add)

                def finish_head(h):
                    # denominator -> reciprocal (after last dacc add)
                    ot_ps, ds_ps, dacc = pend.pop(h)
                    nc.tensor.matmul(ds_ps[0:1, :], ones[:], dacc[:],
                                     start=True, stop=True)
                    rec = sml.tile([1, NQ], DT.float32, tag='rec')
                    nc.vector.reciprocal(rec[:], ds_ps[0:1, :])
                    rech = sml.tile([1, NQ], DT.float16, tag='rech')
                    nc.vector.tensor_copy(rech[:], rec[:])
                    bps = pmm.tile([128, NQ], DT.float32, tag='mm')
                    nc.tensor.matmul(bps[:], onesr[:], rech[:],
                                     start=True, stop=True)
                    binv = sml.tile([128, NQ], DT.float16, tag='binv')
                    nc.scalar.copy(binv[:], bps[:])
                    nc.vector.tensor_tensor(out=OT[h][:], in0=ot_ps[:],
                                            in1=binv[:], op=OP.mult)

                def head_attention(h, kn):
                    ot_ps = pot.tile([128, NQ], DT.float32, tag='ot')
                    ds_ps = pds.tile([128, NQ], DT.float32, tag='ds')
                    dacc = att.tile([128, NQ], DT.float16, tag='dacc')
                    pend[h] = (ot_ps, ds_ps, dacc)
                    ETs = {}
                    ETs[0] = ett.tile([128, 512], DT.float16, tag='et',
                                      name='et0')
                    emit_qk(h, kn, 0, ETs[0])
                    for s in range(NKT):
                        if s + 1 < NKT:
                            ETs[s + 1] = ett.tile([128, 512], DT.float16,
                                                  tag='et', name=f'et{s + 1}')
                            emit_qk(h, kn, s + 1, ETs[s + 1])
                        emit_pv(h, s, ETs.pop(s), ot_ps, dacc)

                kn_next = None
                for h in range(H):
                    if h % 4 == 0:
                        emit_v4_group(h // 4)
                    kn = kn_next if kn_next is not None else emit_knope(h)
                    head_attention(h, kn)
                    # overlap next head's knope with this head's tail
                    if h + 1 < H:
                        kn_next = emit_knope(h + 1)
                    finish_head(h)

                # ---- S7: Wo ----------------------------------------------
                for m in range(NKD):
                    wo = s7w.tile([128, H, 128], DT.float16, tag='wo')
                    nc.sync.dma_start(
                        out=wo[:],
                        in_=wo_d[m].rearrange("p (k c) -> p k c", k=H))
                    ps = pmm.tile([128, NQ], DT.float32, tag='mm')
                    for h in range(H):
                        nc.tensor.matmul(ps[:], wo[:, h, :], OT[h][:],
                                         start=(h == 0), stop=(h == H - 1))
                    o = s7o.tile([128, NQ], DT.float32, tag='o')
                    nc.scalar.copy(o[:], ps[:])
                    nc.sync.dma_start(out=out_d[m, :, :], in_=o[:])

    _split_multiwait_instructions(nc)
    return nc


# ---------------------------------------------------------------------------
# host-side input preparation
# ---------------------------------------------------------------------------

def _ktile(x, dtype=F16):
    """[K, N] -> [K/128, 128, N] contiguous row-block tiling."""
    k, n2 = x.shape
    return np.ascontiguousarray(x.reshape(k // 128, 128, n2).astype(dtype))


def _mtile(x, mt, dtype=F16):
    """[K, M] -> [M/mt, 128, (K/128)*mt]: contiguous per-m-tile weight
    layout; slot [m, p, k*mt+c] = x[k*128+p, m*mt+c]."""
    k, mm = x.shape
    r = x.reshape(k // 128, 128, mm // mt, mt).transpose(2, 1, 0, 3)
    return np.ascontiguousarray(
        r.reshape(mm // mt, 128, (k // 128) * mt).astype(dtype))


def _permute_wqu(wqu):
    """Reorder Wq_up columns: [h0..h15 nope (2048) | rope pair-tiles (1024)].

    Pair-tile p (p=0..7) holds heads 2p, 2p+1 as
    [real(32); imag(32); real'(32); imag'(32)] along output rows.
    """
    perm = np.zeros(H * DQK, dtype=np.int64)
    for h in range(H):
        src = h * DQK
        perm[h * NOPE:(h + 1) * NOPE] = np.arange(src, src + NOPE)
        base = H * NOPE + (h // 2) * 128 + (h % 2) * 64
        perm[base:base + 32] = src + NOPE + 2 * np.arange(32)
        perm[base + 32:base + 64] = src + NOPE + 2 * np.arange(32) + 1
    return wqu[:, perm]


def _permute_wkvd(wkvd):
    """Reorder Wkv_down rope cols (last 64) to [real(32) | imag(32)]."""
    out = wkvd.copy()
    rope = wkvd[:, KVLR:]
    out[:, KVLR:KVLR + 32] = rope[:, 0::2]
    out[:, KVLR + 32:] = rope[:, 1::2]
    return out


def _prepare_inputs(inputs):
    hidden = np.asarray(inputs['hidden_states'], np.float32)
    fc = np.asarray(inputs['freqs_cos'], np.float32)
    fs = np.asarray(inputs['freqs_sin'], np.float32)
    wqd = np.asarray(inputs['Wq_down'], np.float32)
    wkvd = _permute_wkvd(np.asarray(inputs['Wkv_down'], np.float32))
    wqu = _permute_wqu(np.asarray(inputs['Wq_up'], np.float32))
    wkvu = np.asarray(inputs['Wkv_up'], np.float32)
    wo = np.asarray(inputs['Wo'], np.float32)

    shared = {
        'wqd': _mtile(wqd, 128), 'wkvd': _ktile(wkvd),
        'wqu': _mtile(wqu, 128),
        'wkvuk': _ktile(wkvu[:, :H * NOPE]),
        'wkvuv': _mtile(wkvu[:, H * NOPE:], 512),
        'wo': _mtile(wo, 128),
    }
    ck = np.ascontiguousarray(fc[:S].T)
    sk = np.ascontiguousarray(fs[:S].T)
    shared['cosk'] = np.concatenate([ck, ck], 0).astype(F16)
    shared['ssink'] = np.concatenate([-sk, sk], 0).astype(F16)

    in_maps = []
    pos_all = []
    kp = np.arange(128)[:, None]
    qf = np.arange(128)[None, :]
    for core in range(8):
        b, c = core // 4, core % 4
        pos = np.concatenate(
            [np.arange((4 * j + c) * 128, (4 * j + c + 1) * 128)
             for j in range(NSTRIP)])
        pos_all.append((b, pos))
        hT = np.ascontiguousarray(hidden[b].T)
        m = dict(shared)
        m['hT'] = _ktile(hT)
        m['hTq'] = _ktile(np.ascontiguousarray(hT[:, pos]))
        cq = fc[pos].T.astype(F16)
        sq = fs[pos].T.astype(F16)
        m['cosq'] = np.ascontiguousarray(np.concatenate([cq, cq, cq, cq], 0))
        m['ssinq'] = np.ascontiguousarray(np.concatenate([-sq, sq, -sq, sq], 0))
        # binm[kp, mm*128+qf]: key s*128+kp valid for query (4*(s//4)+c)*128+qf
        # iff kp <= qf + (c - s%4)*128, with mm = s%4
        bm = np.zeros((128, 4, 128), dtype=F16)
        for mm_ in range(4):
            bm[:, mm_, :] = (kp <= qf + (c - mm_) * 128).astype(F16)
        m['binm'] = np.ascontiguousarray(bm.reshape(128, 512))
        in_maps.append(m)
    return in_maps, pos_all


def kernel(**inputs):
    global _COMPILED
    _apply_tile_patch()
    from concourse.bass_utils import run_bass_kernel_spmd

    if _COMPILED is None:
        _COMPILED = _build_nc()
    nc = _COMPILED

    in_maps, pos_all = _prepare_inputs(inputs)
    res = run_bass_kernel_spmd(nc, in_maps, list(range(8)))

    out = np.zeros((B, S, D), dtype=np.float32)
    for core in range(8):
        b, pos = pos_all[core]
        ft = res.results[core]['outT']
        out[b, pos, :] = ft.reshape(D, NQ).T
    return out
